# revision 28
# baseline (speedup 1.0000x reference)
"""Single-head causal attention on 8 Trainium2 NeuronCores (Bass/Tile).

Problem: x [4, 4096, 1024] f32, Wq/Wk/Wv [64, 1024] f32 ->
         softmax(causal(q k^T * H^-0.5)) v   -> [4, 4096, 64] f32

Sharding: core = (batch b, parity p), b = core//2, p = core%2. Each core owns
the global 128-wide query tiles g = 2j+p (j=0..15) of its batch -- the parity
interleave balances causal work AND keeps the compiled graph identical across
all 8 cores (SPMD: one NEFF). All parity differences live in host-prepared
data, never in the graph:

  * x arrives as a per-core SHIFTED transpose xt [C, T] whose 128-col key
    blocks are: p=0 -> [zeros | x.T blocks 0..30], p=1 -> [x.T blocks 0..31].
    In this local key space both parities share identical causal geometry:
    local key block k' is fully visible to local query tile r of chunk ch
    (global q-tile g = 8ch+2r+p) iff k' <= 8ch+2r, diagonal (lower-tri mask)
    at k' = 8ch+2r+1, fully masked beyond -- parity-free.
  * The zero-pad block contributes exp(0)*128 = 128 to every softmax
    denominator of p=0 cores; a host-supplied per-core constant (dbias)
    subtracts it exactly before the reciprocal.
  * Queries live in the odd local key blocks (orig g = 2j+p <-> k' = 2j+1),
    so Q projection reads a strided view of the same resident xt -- no
    second copy of x is transferred.

Device pipeline (bf16 matmuls, f32 PSUM accumulation):
  1. xt fully SBUF-resident via 32 per-(c-tile, wave) DMAs (2 KB lines).
  2. Q^T projection with duplicated weights [Wq.T|Wq.T]; fused [K^T;V^T]
     projection; K^T lands on PSUM rows 64:128 and is repartitioned to rows
     0:64 via SBUF->SBUF DMA (GpSimd ring, off the main DMA stream); V^T ->
     V via PE transposes with a ones-column so the softmax denominator falls
     out of the AV matmul (row 64 of O^T).
  3. Per 512-wide q-chunk ch (extent 8ch+8 k-tiles, in pairs): S^T tiles
     [128k, 512q] on PE -> exp on ScalarE (scale=0.125 folded) -> lower-tri
     mask multiply on the diagonal blocks (DVE) -> AV accumulation. Fully
     masked left col-blocks are suffix-sliced out of S^T/exp/AV. Next
     phase's projection work is drip-fed between groups so ScalarE (the
     critical engine) never starves.
  4. Epilogue per chunk: PE transpose [65,128]->[128,65], subtract dbias,
     reciprocal, scale, DMA out.
"""
import os

import numpy as np
import ml_dtypes

import concourse.bass as bass
import concourse.mybir as mybir
import concourse.tile as tile
from concourse import bacc
from concourse.bass_utils import run_bass_kernel_spmd
from concourse.masks import make_identity

P = 128
B, T, C, H = 4, 4096, 1024, 64
TQ = T // 2          # queries per core
CH = 512             # q-chunk width
NCH = TQ // CH       # 4 q-chunks
CT = C // P          # 8 contraction tiles
TC = T // CH         # 8 t-chunks for K/V proj
NKT = T // P         # 32 k-tiles
GROUP = 1            # k-tiles per exp tile (depth-4 pipeline)
N_CORES = 8

F32 = mybir.dt.float32
BF16 = mybir.dt.bfloat16
Exp = mybir.ActivationFunctionType.Exp
MULT = mybir.AluOpType.mult
SUB = mybir.AluOpType.subtract

LAST_EXEC_TIME_NS = None
_COMPILED = None


def _build_graph():
    nc = bacc.Bacc("TRN2", target_bir_lowering=False, debug=False,
                   num_devices=N_CORES)
    xt = nc.dram_tensor("xt", [C, T], BF16, kind="ExternalInput").ap()
    wqq = nc.dram_tensor("wqq", [C, P], BF16, kind="ExternalInput").ap()
    wkv = nc.dram_tensor("wkv", [C, P], BF16, kind="ExternalInput").ap()
    mtri = nc.dram_tensor("mtri", [P, P], BF16, kind="ExternalInput").ap()
    dbias = nc.dram_tensor("dbias", [P, 1], F32, kind="ExternalInput").ap()
    y = nc.dram_tensor("y", [TQ, H], F32, kind="ExternalOutput").ap()

    xt_r = xt.rearrange("(co p) t -> p co t", p=P)     # [128, 8, 4096]
    xt_qr = xt.rearrange("(co p) (hb two q) -> p co hb two q",
                         p=P, two=2, q=P)               # [128, 8, 16, 2, 128]
    wqq_r = wqq.rearrange("(co p) m -> p co m", p=P)   # [128, 8, 128]
    wkv_r = wkv.rearrange("(co p) m -> p co m", p=P)

    with tile.TileContext(nc) as tc:
        with (
            tc.tile_pool(name="const", bufs=1) as const,
            tc.tile_pool(name="ssb", bufs=6) as sspool,
            tc.tile_pool(name="epi", bufs=2) as epool,
            tc.tile_pool(name="pproj", bufs=1, space="PSUM") as ppool,
            tc.tile_pool(name="ps", bufs=4, space="PSUM") as spool,
            tc.tile_pool(name="po", bufs=1, space="PSUM") as opool,
            tc.tile_pool(name="pt", bufs=2, space="PSUM") as tpool,
        ):
            # ---- constants ----
            wqq_sb = const.tile([P, CT, P], BF16, name="wqq_sb")
            wkv_sb = const.tile([P, CT, P], BF16, name="wkv_sb")
            mask_sb = const.tile([P, P], BF16, name="mask_sb")
            dbias_sb = const.tile([P, 1], F32, name="dbias_sb")
            ident16 = const.tile([P, P], BF16, name="ident16")
            ident32 = const.tile([P, P], F32, name="ident32")
            scratch = const.tile([P, 1], F32, name="scratch")
            nc.gpsimd.dma_start(wqq_sb[:], wqq_r)
            nc.gpsimd.dma_start(wkv_sb[:], wkv_r)
            nc.gpsimd.dma_start(mask_sb[:], mtri)
            nc.gpsimd.dma_start(dbias_sb[:], dbias)
            # preload the exp table set immediately (scratch <- exp(0))
            nc.vector.memset(scratch[:], 0.0)
            nc.scalar.activation(scratch[:], scratch[:], Exp)
            make_identity(nc, ident16[:])
            make_identity(nc, ident32[:])

            # ---- resident x ----
            xt_sb = const.tile([P, CT, T], BF16, name="xt_sb")
            # odd local key blocks hold this core's query tokens
            xt_q = xt_sb.rearrange("p co (hb two q) -> p co hb two q",
                                   two=2, q=P)          # [128, 8, 16, 2, 128]

            # ---- persistent activations ----
            qt_sb = const.tile([P, TQ], BF16, name="qt_sb")      # Q^T dup rows
            kt_sb = const.tile([P, T], BF16, name="kt_sb")       # K^T top, zero bottom
            kstage = const.tile([P, T], BF16, name="kstage")     # K^T at rows 64:128
            vt_sb = const.tile([64, T], BF16, name="vt_sb")      # V^T
            v_sb = const.tile([P, NKT, H + 1], BF16, name="v_sb")  # V tiles + ones

            nc.gpsimd.memset(kt_sb[64:128, :], 0.0)
            nc.gpsimd.memset(v_sb[:, :, H:H + 1], 1.0)

            # ---- DMA schedule ----
            # xt arrives in 4 consumption-ordered 1024-col waves, one DMA per
            # (c-tile, wave): 256 KB contiguous-flat-range transfers (2 KB
            # lines). Rationale: a single dma_start already fans out across
            # all 16 SDMA engines and HWDGE executes FIFO per ring, so few
            # big transfers beat many small ones; per-c granularity keeps the
            # write footprints contiguous so Tile's dependency tracker stays
            # precise (multi-co strided writes make every later reader wait);
            # everything goes on the sync ring only (dma_start issues on the
            # scalar ring can stall the ScalarE exp stream via backpressure).
            # One 2 MB multi-co DMA per 1024-col wave (big transfers run
            # ~340 GB/s; a single dma_start fans across all 16 SDMA engines).
            # A multi-co write's flat footprint makes EVERY later-emitted
            # xt reader wait on it, so each wave is issued just-in-time:
            # wave w right before the phase that consumes it -- readers sit
            # between their wave's issue and the next one's.
            def issue_wave(w):
                nc.sync.dma_start(xt_sb[:, :, bass.ts(w, 2 * CH)],
                                  xt_r[:, :, bass.ts(w, 2 * CH)])

            issue_wave(0)

            # ---- projection work units (drip-fed between attention groups) --
            def q_proj_units(qc):
                ps = ppool.tile([P, CH], F32, tag="ps_proj")
                for c in range(CT):
                    yield lambda c=c, ps=ps: nc.tensor.matmul(
                        ps[:], lhsT=wqq_sb[:, c, :],
                        rhs=xt_q[:, c, bass.ts(qc, 4), 1, :],
                        start=(c == 0), stop=(c == CT - 1))
                yield lambda ps=ps: nc.vector.tensor_copy(
                    qt_sb[:, bass.ts(qc, CH)], ps[:])

            def kv_proj_units(t_i):
                ps = ppool.tile([P, CH], F32, tag="ps_proj")
                for c in range(CT):
                    yield lambda c=c, ps=ps: nc.tensor.matmul(
                        ps[:], lhsT=wkv_sb[:, c, :],
                        rhs=xt_sb[:, c, bass.ts(t_i, CH)],
                        start=(c == 0), stop=(c == CT - 1))

                def evac_k(ps=ps):
                    nc.vector.tensor_copy(kstage[64:128, bass.ts(t_i, CH)],
                                          ps[64:128, :])
                    nc.gpsimd.dma_start(kt_sb[0:64, bass.ts(t_i, CH)],
                                        kstage[64:128, bass.ts(t_i, CH)])
                yield evac_k
                yield lambda ps=ps: nc.vector.tensor_copy(
                    vt_sb[:, bass.ts(t_i, CH)], ps[0:64, :])
                for j in range(CH // P):
                    def vtile(j=j):
                        kt = t_i * (CH // P) + j
                        pt = tpool.tile([P, P], BF16, tag="tr")
                        nc.tensor.transpose(pt[:, 0:64], vt_sb[:, bass.ts(kt, P)],
                                            ident16[0:64, 0:64])
                        nc.vector.tensor_copy(v_sb[:, kt, 0:H], pt[:, 0:64])
                    yield vtile

            def epilogue_units(ch, po):
                # po is fully accumulated; evacuate it promptly (releases the
                # single po slot), then normalize+store subtile by subtile.
                osb = epool.tile([H + 1, CH], F32, tag="osb")
                yield lambda: nc.vector.tensor_copy(osb[:], po[0:H + 1, :])
                for s in range(CH // P):
                    def sub(s=s):
                        pt2 = tpool.tile([P, P], F32, tag="tr")
                        nc.tensor.transpose(pt2[:, 0:H + 1], osb[:, bass.ts(s, P)],
                                            ident32[0:H + 1, 0:H + 1])
                        den = epool.tile([P, 1], F32, tag="den")
                        nc.vector.tensor_tensor(den[:], pt2[:, H:H + 1],
                                                dbias_sb[:], SUB)
                        rec = epool.tile([P, 1], F32, tag="rec")
                        nc.vector.reciprocal(rec[:], den[:])
                        ot = epool.tile([P, H], F32, tag="ot")
                        nc.vector.tensor_scalar_mul(ot[:], pt2[:, 0:H], rec[:])
                        nc.gpsimd.dma_start(y[bass.ds(ch * CH + s * P, P), :], ot[:])
                    yield sub

            def phase_units(phase):
                # Q first: it gates the next chunk's very first S^T; the KV
                # chunks' V-tiles are only needed by progressively later AVs
                yield from q_proj_units(phase)
                yield from kv_proj_units(2 * phase)
                yield from kv_proj_units(2 * phase + 1)

            # ---- attention: flat software pipeline, 4-deep lookahead ----
            # One k-tile per exp op ([128k, 512q] -> 1 PSUM bank): 4 score
            # tiles in flight absorb the multi-us projection bursts that sit
            # in PE's in-order stream, so ScalarE (the critical engine) never
            # starves. The extra per-op ACT overhead is far cheaper than the
            # pipeline stalls it removes.
            DEPTH = 4

            def n_groups_of(ch):
                return 8 * ch + 8

            def emit_st(ch, kt):
                # col-blocks with k'-8ch-2r >= 2 are fully masked: suffix-
                # slice them out of S^T, exp and AV.
                r0 = max(0, (kt - 8 * ch) // 2)
                ps_s = spool.tile([P, CH], F32, name="ps_s")
                s_sb = sspool.tile([P, CH], BF16, tag="s_sb")
                nc.tensor.matmul(
                    ps_s[:, r0 * P:CH],
                    lhsT=kt_sb[:, bass.ts(kt, P)],
                    rhs=qt_sb[:, ch * CH + r0 * P: (ch + 1) * CH],
                    start=True, stop=True)
                return ps_s, s_sb, r0

            # pre-work: Q0 + KV0 cover the first four k-tiles (Q0 first:
            # its query-block wave is the first DMA to land)
            for u in q_proj_units(0):
                u()
            for u in kv_proj_units(0):
                u()
            flat = [(ch, kt) for ch in range(NCH) for kt in range(n_groups_of(ch))]
            pending = {}
            for k in range(DEPTH):
                pending[flat[k]] = emit_st(*flat[k])
            for u in kv_proj_units(1):
                u()
            issue_wave(1)
            carry = []
            po = None
            feeder = iter(())
            per_group = 1
            for i, (ch, kt) in enumerate(flat):
                ext = 8 * ch + 8
                n_groups = n_groups_of(ch)
                if kt == 0:
                    if ch >= 1 and ch <= 2:
                        issue_wave(ch + 1)
                    po = opool.tile([P, CH], F32, name="po")
                    feeder = iter(carry + (list(phase_units(ch + 1))
                                           if ch + 1 < NCH else []))
                    carry = []
                    per_group = max(1, 35 // max(1, n_groups - DEPTH) + 1)
                if i + DEPTH < len(flat):
                    pending[flat[i + DEPTH]] = emit_st(*flat[i + DEPTH])
                ps_s, s_sb, r0 = pending.pop((ch, kt))
                nc.scalar.activation(s_sb[:, r0 * P:CH], ps_s[:, r0 * P:CH],
                                     Exp, scale=0.125)
                d = kt - 8 * ch
                if d >= 1 and d % 2 == 1:  # diagonal block at r=(d-1)/2
                    r = (d - 1) // 2
                    blk = s_sb[:, r * P:(r + 1) * P]
                    nc.vector.tensor_tensor(blk, blk, mask_sb[:], MULT)
                nc.tensor.matmul(po[0:H + 1, r0 * P:CH],
                                 lhsT=v_sb[:, kt, :],
                                 rhs=s_sb[:, r0 * P:CH],
                                 start=(kt == 0), stop=(kt == ext - 1))
                for _ in range(per_group):
                    u = next(feeder, None)
                    if u is None:
                        break
                    u()
                if kt == n_groups - 1:
                    for u in feeder:
                        u()
                    # epilogue: evacuate po now; per-subtile normalize/store
                    # drips into the next chunk (or runs inline for the last)
                    epi = epilogue_units(ch, po)
                    next(epi)()  # osb evacuation (releases the po slot)
                    if ch + 1 < NCH:
                        carry = list(epi)
                    else:
                        for u in epi:
                            u()

    nc.compile()
    return nc


def _shard_inputs(x, Wq, Wk, Wv):
    bf = ml_dtypes.bfloat16
    tri = np.tril(np.ones((P, P), dtype=np.float32)).T  # [kk,qq]=1 iff kk<=qq
    wqq = np.concatenate([Wq.T, Wq.T], axis=1).astype(bf)
    wkv = np.concatenate([Wv.T, Wk.T], axis=1).astype(bf)
    mtri = tri.astype(bf)
    in_maps = []
    for core in range(N_CORES):
        b, p = core // 2, core % 2
        if p == 0:
            # [zeros | blocks 0..30]
            xt_full = np.concatenate(
                [np.zeros((P, C), dtype=np.float32), x[b][:T - P]], axis=0).T
        else:
            xt_full = x[b].T
        xt_core = np.ascontiguousarray(xt_full).astype(bf)
        db = np.full((P, 1), 128.0 if p == 0 else 0.0, dtype=np.float32)
        in_maps.append({"xt": xt_core, "wqq": wqq, "wkv": wkv,
                        "mtri": mtri, "dbias": db})
    return in_maps


def _unshard(results):
    y = np.zeros((B, T, H), dtype=np.float32)
    for core in range(N_CORES):
        b, p = core // 2, core % 2
        yc = results[core]["y"]
        for j in range(16):
            g = 2 * j + p
            y[b, P * g:P * g + P] = yc[P * j:P * j + P]
    return y


def kernel(x, Wq, Wk, Wv):
    global LAST_EXEC_TIME_NS, _COMPILED
    x = np.asarray(x, dtype=np.float32)
    Wq = np.asarray(Wq, dtype=np.float32)
    Wk = np.asarray(Wk, dtype=np.float32)
    Wv = np.asarray(Wv, dtype=np.float32)

    if _COMPILED is None:
        _COMPILED = _build_graph()
    nc = _COMPILED

    in_maps = _shard_inputs(x, Wq, Wk, Wv)
    kwargs = {}
    if os.environ.get("ATTN_TRACE"):
        kwargs["trace"] = True
        if os.environ.get("ATTN_TRACE_DIR"):
            kwargs["tmpdir"] = os.environ["ATTN_TRACE_DIR"]
    res = run_bass_kernel_spmd(nc, in_maps, core_ids=list(range(N_CORES)), **kwargs)
    LAST_EXEC_TIME_NS = res.exec_time_ns
    return _unshard(res.results)


# revision 29
# speedup vs baseline: 1.0148x; 1.0148x over previous
"""Single-head causal attention on 8 Trainium2 NeuronCores (Bass/Tile).

Problem: x [4, 4096, 1024] f32, Wq/Wk/Wv [64, 1024] f32 ->
         softmax(causal(q k^T * H^-0.5)) v   -> [4, 4096, 64] f32

Sharding: core = (batch b, parity p), b = core//2, p = core%2. Each core owns
the global 128-wide query tiles g = 2j+p (j=0..15) of its batch -- the parity
interleave balances causal work AND keeps the compiled graph identical across
all 8 cores (SPMD: one NEFF). All parity differences live in host-prepared
data, never in the graph:

  * x arrives as a per-core SHIFTED transpose xt [C, T] whose 128-col key
    blocks are: p=0 -> [zeros | x.T blocks 0..30], p=1 -> [x.T blocks 0..31].
    In this local key space both parities share identical causal geometry:
    local key block k' is fully visible to local query tile r of chunk ch
    (global q-tile g = 8ch+2r+p) iff k' <= 8ch+2r, diagonal (lower-tri mask)
    at k' = 8ch+2r+1, fully masked beyond -- parity-free.
  * The zero-pad block contributes exp(0)*128 = 128 to every softmax
    denominator of p=0 cores; a host-supplied per-core constant (dbias)
    subtracts it exactly before the reciprocal.
  * Queries live in the odd local key blocks (orig g = 2j+p <-> k' = 2j+1),
    so Q projection reads a strided view of the same resident xt -- no
    second copy of x is transferred.

Device pipeline (bf16 matmuls, f32 PSUM accumulation):
  1. xt fully SBUF-resident via 32 per-(c-tile, wave) DMAs (2 KB lines).
  2. Q^T projection with duplicated weights [Wq.T|Wq.T]; fused [K^T;V^T]
     projection; K^T lands on PSUM rows 64:128 and is repartitioned to rows
     0:64 via SBUF->SBUF DMA (GpSimd ring, off the main DMA stream); V^T ->
     V via PE transposes with a ones-column so the softmax denominator falls
     out of the AV matmul (row 64 of O^T).
  3. Per 512-wide q-chunk ch (extent 8ch+8 k-tiles, in pairs): S^T tiles
     [128k, 512q] on PE -> exp on ScalarE (scale=0.125 folded) -> lower-tri
     mask multiply on the diagonal blocks (DVE) -> AV accumulation. Fully
     masked left col-blocks are suffix-sliced out of S^T/exp/AV. Next
     phase's projection work is drip-fed between groups so ScalarE (the
     critical engine) never starves.
  4. Epilogue per chunk: PE transpose [65,128]->[128,65], subtract dbias,
     reciprocal, scale, DMA out.
"""
import os

import numpy as np
import ml_dtypes

import concourse.bass as bass
import concourse.mybir as mybir
import concourse.tile as tile
from concourse import bacc
from concourse.bass_utils import run_bass_kernel_spmd
from concourse.masks import make_identity

P = 128
B, T, C, H = 4, 4096, 1024, 64
TQ = T // 2          # queries per core
CH = 512             # q-chunk width
NCH = TQ // CH       # 4 q-chunks
CT = C // P          # 8 contraction tiles
TC = T // CH         # 8 t-chunks for K/V proj
NKT = T // P         # 32 k-tiles
GROUP = 1            # k-tiles per exp tile (depth-4 pipeline)
N_CORES = 8

F32 = mybir.dt.float32
BF16 = mybir.dt.bfloat16
Exp = mybir.ActivationFunctionType.Exp
MULT = mybir.AluOpType.mult
SUB = mybir.AluOpType.subtract

LAST_EXEC_TIME_NS = None
_COMPILED = None


def _build_graph():
    nc = bacc.Bacc("TRN2", target_bir_lowering=False, debug=False,
                   num_devices=N_CORES)
    xt = nc.dram_tensor("xt", [C, T], BF16, kind="ExternalInput").ap()
    wqq = nc.dram_tensor("wqq", [C, P], BF16, kind="ExternalInput").ap()
    wkv = nc.dram_tensor("wkv", [C, P], BF16, kind="ExternalInput").ap()
    mtri = nc.dram_tensor("mtri", [P, P], BF16, kind="ExternalInput").ap()
    dbias = nc.dram_tensor("dbias", [P, 1], F32, kind="ExternalInput").ap()
    y = nc.dram_tensor("y", [TQ, H], F32, kind="ExternalOutput").ap()

    xt_r = xt.rearrange("(co p) t -> p co t", p=P)     # [128, 8, 4096]
    wqq_r = wqq.rearrange("(co p) m -> p co m", p=P)   # [128, 8, 128]
    wkv_r = wkv.rearrange("(co p) m -> p co m", p=P)

    with tile.TileContext(nc) as tc:
        with (
            tc.tile_pool(name="const", bufs=1) as const,
            tc.tile_pool(name="ssb", bufs=6) as sspool,
            tc.tile_pool(name="epi", bufs=2) as epool,
            tc.tile_pool(name="pproj", bufs=1, space="PSUM") as ppool,
            tc.tile_pool(name="ps", bufs=4, space="PSUM") as spool,
            tc.tile_pool(name="po", bufs=1, space="PSUM") as opool,
            tc.tile_pool(name="pt", bufs=2, space="PSUM") as tpool,
        ):
            # ---- constants ----
            wqq_sb = const.tile([P, CT, P], BF16, name="wqq_sb")
            wkv_sb = const.tile([P, CT, P], BF16, name="wkv_sb")
            mask_sb = const.tile([P, P], BF16, name="mask_sb")
            dbias_sb = const.tile([P, 1], F32, name="dbias_sb")
            ident16 = const.tile([P, P], BF16, name="ident16")
            ident32 = const.tile([P, P], F32, name="ident32")
            scratch = const.tile([P, 1], F32, name="scratch")
            nc.gpsimd.dma_start(wqq_sb[:], wqq_r)
            nc.gpsimd.dma_start(wkv_sb[:], wkv_r)
            nc.gpsimd.dma_start(mask_sb[:], mtri)
            nc.gpsimd.dma_start(dbias_sb[:], dbias)
            # preload the exp table set immediately (scratch <- exp(0))
            nc.vector.memset(scratch[:], 0.0)
            nc.scalar.activation(scratch[:], scratch[:], Exp)
            make_identity(nc, ident16[:])
            make_identity(nc, ident32[:])

            # ---- resident x ----
            xt_sb = const.tile([P, CT, T], BF16, name="xt_sb")
            # odd local key blocks hold this core's query tokens
            xt_q = xt_sb.rearrange("p co (hb two q) -> p co hb two q",
                                   two=2, q=P)          # [128, 8, 16, 2, 128]

            # ---- persistent activations ----
            qt_sb = const.tile([P, TQ], BF16, name="qt_sb")      # Q^T dup rows
            kt_sb = const.tile([P, T], BF16, name="kt_sb")       # K^T top, zero bottom
            kstage = const.tile([P, T], BF16, name="kstage")     # K^T at rows 64:128
            vt_sb = const.tile([64, T], BF16, name="vt_sb")      # V^T
            v_sb = const.tile([P, NKT, H + 1], BF16, name="v_sb")  # V tiles + ones

            nc.gpsimd.memset(kt_sb[64:128, :], 0.0)
            nc.gpsimd.memset(v_sb[:, :, H:H + 1], 1.0)

            # ---- DMA schedule ----
            # xt arrives in 4 consumption-ordered 1024-col waves, one DMA per
            # (c-tile, wave): 256 KB contiguous-flat-range transfers (2 KB
            # lines). Rationale: a single dma_start already fans out across
            # all 16 SDMA engines and HWDGE executes FIFO per ring, so few
            # big transfers beat many small ones; per-c granularity keeps the
            # write footprints contiguous so Tile's dependency tracker stays
            # precise (multi-co strided writes make every later reader wait);
            # everything goes on the sync ring only (dma_start issues on the
            # scalar ring can stall the ScalarE exp stream via backpressure).
            for c in range(CT):
                nc.sync.dma_start(xt_sb[:, c, 0:CH], xt_r[:, c, 0:CH])
            for c in range(CT):
                nc.sync.dma_start(xt_sb[:, c, CH:2 * CH], xt_r[:, c, CH:2 * CH])
            for w in range(1, NCH):
                for c in range(CT):
                    nc.sync.dma_start(xt_sb[:, c, bass.ts(w, 2 * CH)],
                                      xt_r[:, c, bass.ts(w, 2 * CH)])

            # ---- projection work units (drip-fed between attention groups) --
            def q_proj_units(qc):
                ps = ppool.tile([P, CH], F32, tag="ps_proj")
                for c in range(CT):
                    yield lambda c=c, ps=ps: nc.tensor.matmul(
                        ps[:], lhsT=wqq_sb[:, c, :],
                        rhs=xt_q[:, c, bass.ts(qc, 4), 1, :],
                        start=(c == 0), stop=(c == CT - 1))
                yield lambda ps=ps: nc.vector.tensor_copy(
                    qt_sb[:, bass.ts(qc, CH)], ps[:])

            def kv_proj_units(t_i):
                ps = ppool.tile([P, CH], F32, tag="ps_proj")
                for c in range(CT):
                    yield lambda c=c, ps=ps: nc.tensor.matmul(
                        ps[:], lhsT=wkv_sb[:, c, :],
                        rhs=xt_sb[:, c, bass.ts(t_i, CH)],
                        start=(c == 0), stop=(c == CT - 1))

                def evac_k(ps=ps):
                    nc.vector.tensor_copy(kstage[64:128, bass.ts(t_i, CH)],
                                          ps[64:128, :])
                    nc.gpsimd.dma_start(kt_sb[0:64, bass.ts(t_i, CH)],
                                        kstage[64:128, bass.ts(t_i, CH)])
                yield evac_k
                yield lambda ps=ps: nc.vector.tensor_copy(
                    vt_sb[:, bass.ts(t_i, CH)], ps[0:64, :])
                for j in range(CH // P):
                    def vtile(j=j):
                        kt = t_i * (CH // P) + j
                        pt = tpool.tile([P, P], BF16, tag="tr")
                        nc.tensor.transpose(pt[:, 0:64], vt_sb[:, bass.ts(kt, P)],
                                            ident16[0:64, 0:64])
                        nc.vector.tensor_copy(v_sb[:, kt, 0:H], pt[:, 0:64])
                    yield vtile

            def epilogue_units(ch, po):
                # po is fully accumulated; evacuate it promptly (releases the
                # single po slot), then normalize+store subtile by subtile.
                osb = epool.tile([H + 1, CH], F32, tag="osb")
                yield lambda: nc.vector.tensor_copy(osb[:], po[0:H + 1, :])
                for s in range(CH // P):
                    def sub(s=s):
                        pt2 = tpool.tile([P, P], F32, tag="tr")
                        nc.tensor.transpose(pt2[:, 0:H + 1], osb[:, bass.ts(s, P)],
                                            ident32[0:H + 1, 0:H + 1])
                        den = epool.tile([P, 1], F32, tag="den")
                        nc.vector.tensor_tensor(den[:], pt2[:, H:H + 1],
                                                dbias_sb[:], SUB)
                        rec = epool.tile([P, 1], F32, tag="rec")
                        nc.vector.reciprocal(rec[:], den[:])
                        ot = epool.tile([P, H], F32, tag="ot")
                        nc.vector.tensor_scalar_mul(ot[:], pt2[:, 0:H], rec[:])
                        nc.gpsimd.dma_start(y[bass.ds(ch * CH + s * P, P), :], ot[:])
                    yield sub

            def phase_units(phase):
                # Q first: it gates the next chunk's very first S^T; the KV
                # chunks' V-tiles are only needed by progressively later AVs
                yield from q_proj_units(phase)
                yield from kv_proj_units(2 * phase)
                yield from kv_proj_units(2 * phase + 1)

            # ---- attention: flat software pipeline, 4-deep lookahead ----
            # One k-tile per exp op ([128k, 512q] -> 1 PSUM bank): 4 score
            # tiles in flight absorb the multi-us projection bursts that sit
            # in PE's in-order stream, so ScalarE (the critical engine) never
            # starves. The extra per-op ACT overhead is far cheaper than the
            # pipeline stalls it removes.
            DEPTH = 4

            def n_groups_of(ch):
                return 8 * ch + 8

            def emit_st(ch, kt):
                # col-blocks with k'-8ch-2r >= 2 are fully masked: suffix-
                # slice them out of S^T, exp and AV.
                r0 = max(0, (kt - 8 * ch) // 2)
                ps_s = spool.tile([P, CH], F32, name="ps_s")
                s_sb = sspool.tile([P, CH], BF16, tag="s_sb")
                nc.tensor.matmul(
                    ps_s[:, r0 * P:CH],
                    lhsT=kt_sb[:, bass.ts(kt, P)],
                    rhs=qt_sb[:, ch * CH + r0 * P: (ch + 1) * CH],
                    start=True, stop=True)
                return ps_s, s_sb, r0

            # pre-work: Q0 + KV0 cover the first four k-tiles
            for u in kv_proj_units(0):
                u()
            for u in q_proj_units(0):
                u()
            flat = [(ch, kt) for ch in range(NCH) for kt in range(n_groups_of(ch))]
            pending = {}
            for k in range(DEPTH):
                pending[flat[k]] = emit_st(*flat[k])
            for u in kv_proj_units(1):
                u()
            carry = []
            po = None
            feeder = iter(())
            per_group = 1
            for i, (ch, kt) in enumerate(flat):
                ext = 8 * ch + 8
                n_groups = n_groups_of(ch)
                if kt == 0:
                    po = opool.tile([P, CH], F32, name="po")
                    feeder = iter(carry + (list(phase_units(ch + 1))
                                           if ch + 1 < NCH else []))
                    carry = []
                    per_group = max(1, 35 // max(1, n_groups - DEPTH) + 1)
                if i + DEPTH < len(flat):
                    pending[flat[i + DEPTH]] = emit_st(*flat[i + DEPTH])
                ps_s, s_sb, r0 = pending.pop((ch, kt))
                nc.scalar.activation(s_sb[:, r0 * P:CH], ps_s[:, r0 * P:CH],
                                     Exp, scale=0.125)
                d = kt - 8 * ch
                if d >= 1 and d % 2 == 1:  # diagonal block at r=(d-1)/2
                    r = (d - 1) // 2
                    blk = s_sb[:, r * P:(r + 1) * P]
                    nc.vector.tensor_tensor(blk, blk, mask_sb[:], MULT)
                nc.tensor.matmul(po[0:H + 1, r0 * P:CH],
                                 lhsT=v_sb[:, kt, :],
                                 rhs=s_sb[:, r0 * P:CH],
                                 start=(kt == 0), stop=(kt == ext - 1))
                for _ in range(per_group):
                    u = next(feeder, None)
                    if u is None:
                        break
                    u()
                if kt == n_groups - 1:
                    for u in feeder:
                        u()
                    # epilogue: evacuate po now; per-subtile normalize/store
                    # drips into the next chunk (or runs inline for the last)
                    epi = epilogue_units(ch, po)
                    next(epi)()  # osb evacuation (releases the po slot)
                    if ch + 1 < NCH:
                        carry = list(epi)
                    else:
                        for u in epi:
                            u()

    nc.compile()
    return nc


def _shard_inputs(x, Wq, Wk, Wv):
    bf = ml_dtypes.bfloat16
    tri = np.tril(np.ones((P, P), dtype=np.float32)).T  # [kk,qq]=1 iff kk<=qq
    wqq = np.concatenate([Wq.T, Wq.T], axis=1).astype(bf)
    wkv = np.concatenate([Wv.T, Wk.T], axis=1).astype(bf)
    mtri = tri.astype(bf)
    in_maps = []
    for core in range(N_CORES):
        b, p = core // 2, core % 2
        if p == 0:
            # [zeros | blocks 0..30]
            xt_full = np.concatenate(
                [np.zeros((P, C), dtype=np.float32), x[b][:T - P]], axis=0).T
        else:
            xt_full = x[b].T
        xt_core = np.ascontiguousarray(xt_full).astype(bf)
        db = np.full((P, 1), 128.0 if p == 0 else 0.0, dtype=np.float32)
        in_maps.append({"xt": xt_core, "wqq": wqq, "wkv": wkv,
                        "mtri": mtri, "dbias": db})
    return in_maps


def _unshard(results):
    y = np.zeros((B, T, H), dtype=np.float32)
    for core in range(N_CORES):
        b, p = core // 2, core % 2
        yc = results[core]["y"]
        for j in range(16):
            g = 2 * j + p
            y[b, P * g:P * g + P] = yc[P * j:P * j + P]
    return y


def kernel(x, Wq, Wk, Wv):
    global LAST_EXEC_TIME_NS, _COMPILED
    x = np.asarray(x, dtype=np.float32)
    Wq = np.asarray(Wq, dtype=np.float32)
    Wk = np.asarray(Wk, dtype=np.float32)
    Wv = np.asarray(Wv, dtype=np.float32)

    if _COMPILED is None:
        _COMPILED = _build_graph()
    nc = _COMPILED

    in_maps = _shard_inputs(x, Wq, Wk, Wv)
    kwargs = {}
    if os.environ.get("ATTN_TRACE"):
        kwargs["trace"] = True
        if os.environ.get("ATTN_TRACE_DIR"):
            kwargs["tmpdir"] = os.environ["ATTN_TRACE_DIR"]
    res = run_bass_kernel_spmd(nc, in_maps, core_ids=list(range(N_CORES)), **kwargs)
    LAST_EXEC_TIME_NS = res.exec_time_ns
    return _unshard(res.results)
